# revision 43
# baseline (speedup 1.0000x reference)
"""Causal self-attention (B=4, T=2048, C=1024, H=16) on 8 TRN2 NeuronCores.

Sharding: core c handles batch b = c//2 and heads [8*(c%2), 8*(c%2)+8).
Each core computes the qkv projection for its 8 heads, flash-style causal
attention, and a partial output projection (its heads' slice of W_out rows).
Host sums the two partials per batch and adds the bias terms (v/out biases
are affine in the output because softmax rows sum to 1).

Schedule (single fused stream, tensor engine never drains):
  prefix   x arrives k-tile by k-tile; v(t0..3)/q(m0)/k(m0) projections run
           k-outer, pipelined against the DMA.
  steady   attention runs per (q-chunk, head-pair). The two heads of a pair
           occupy partitions 0:64 / 64:128, so their K=64 score matmuls
           land on disjoint PE row-groups and execute concurrently (row
           tiling). Remaining qkv projections and the output projection are
           chopped into single-matmul "fill" items, pumped between
           attention ops so the PE stays busy while ACT does the exps.
  denom    the ones-column in v makes the AV matmul emit the softmax
           denominator; 1/den via DVE reciprocal_approx_fast straight from
           PSUM, broadcast on gpsimd, one DVE multiply into aT.
All matmuls bf16 with fp32 PSUM accumulation; output y returned bf16
(partials are summed on host in fp32).
"""
from collections import deque

import numpy as np
import concourse.bass as bass  # noqa: F401  (registers engines)
import concourse.mybir as mybir
import concourse.tile as tile
from concourse import bacc
from concourse.bass_utils import run_bass_kernel_spmd

# problem constants (hardcoded per contract)
B, T, C, H, D = 4, 2048, 1024, 16, 64
NCORES = 8
NH = H // 2          # heads per core = 8
NHP = NH // 2        # head pairs per core = 4
QK = NH * D          # 512 qkv cols per core per q/k/v
SCALE = float(D) ** -0.5
P = 128
NKT = C // P         # 8 contraction tiles for the projections
NMQ = QK // P        # 4 row-tiles of qT/kT
NTT = T // P         # 16 t-blocks
NQC = T // 512       # 4 q-chunks
FP = mybir.dt.float32
BF = mybir.dt.bfloat16
EXP = mybir.ActivationFunctionType.Exp

# fill matmuls per attention pair-step: (before AV flush, after AV flush,
# between the even/odd score groups). Tuned so the fill supply lasts
# through qc=3 (over-pumping early starves the late pairs into HAM
# re-throttle oscillation).
PUMP = {0: (1, 0, 1), 1: (1, 0, 2), 2: (1, 0, 2), 3: (2, 1, 2)}

_NC_CACHE = {}
_LAST_IN_MAPS = None


def build_nc():
    if "nc" in _NC_CACHE:
        return _NC_CACHE["nc"]
    nc = bacc.Bacc(target_bir_lowering=False)
    xT = nc.declare_dram_parameter("xT", [C, T], BF, isOutput=False)
    Wq = nc.declare_dram_parameter("Wq", [NMQ, P, NKT * P], BF, isOutput=False)
    Wk = nc.declare_dram_parameter("Wk", [NMQ, P, NKT * P], BF, isOutput=False)
    Wv = nc.declare_dram_parameter("Wv", [P, NKT * QK], BF, isOutput=False)
    bq = nc.declare_dram_parameter("bq", [QK, 1], FP, isOutput=False)
    bk = nc.declare_dram_parameter("bk", [QK, 1], FP, isOutput=False)
    Wo = nc.declare_dram_parameter("Wo", [QK, C], BF, isOutput=False)
    tri = nc.declare_dram_parameter("tri", [P, P], BF, isOutput=False)
    y = nc.declare_dram_parameter("y", [T, C], BF, isOutput=True)

    with nc.allow_low_precision(reason="bf16 attention"), \
         tile.TileContext(nc) as tc, \
         tc.tile_pool(name="persist", bufs=1) as pers, \
         tc.tile_pool(name="psum", bufs=1, space="PSUM") as pp, \
         tc.tile_pool(name="evict", bufs=1) as ep:

        # ---- persistent tiles
        xsb = [pers.tile([P, T], BF, name=f"x{k}", tag=f"x{k}")
               for k in range(NKT)]
        qT = [pers.tile([P, T], BF, name=f"qT{m}", tag=f"qT{m}")
              for m in range(NMQ)]
        kT = [pers.tile([P, T], BF, name=f"kT{m}", tag=f"kT{m}")
              for m in range(NMQ)]
        aT = [pers.tile([P, T], BF, name=f"aT{m}", tag=f"aT{m}")
              for m in range(NMQ)]
        vsb = [pers.tile([P, NH * 65], BF, name=f"v{t}", tag=f"v{t}")
               for t in range(NTT)]
        wqt = [pers.tile([P, NKT * P], BF, name=f"wq{m}", tag=f"wq{m}")
               for m in range(NMQ)]
        wkt = [pers.tile([P, NKT * P], BF, name=f"wk{m}", tag=f"wk{m}")
               for m in range(NMQ)]
        wvt = pers.tile([P, NKT * QK // P * P], BF, name="wv", tag="wv")
        wot = [pers.tile([P, C], BF, name=f"wo{k}", tag=f"wo{k}")
               for k in range(NMQ)]
        trit = pers.tile([P, P], BF, name="trit", tag="trit")
        bqt = pers.tile([P, NMQ], FP, name="bqt", tag="bqt")
        bkt = pers.tile([P, NMQ], FP, name="bkt", tag="bkt")
        ones64 = pers.tile([1, 64], FP, name="ones64", tag="ones64")

        # ---- DMAs, in the order the prefix consumes them: wv/x chunk k
        # arrive together so the k-outer v matmuls start within ~2us.
        nc.sync.dma_start(trit, tri.ap())
        nc.sync.dma_start(bqt, bq.ap().rearrange("(m p) o -> p (m o)", p=P))
        nc.sync.dma_start(bkt, bk.ap().rearrange("(m p) o -> p (m o)", p=P))
        nc.sync.dma_start(wqt[0], Wq.ap()[0])
        nc.sync.dma_start(wkt[0], Wk.ap()[0])
        for k in range(NKT):
            nc.sync.dma_start(wvt[:, k * QK:(k + 1) * QK],
                              Wv.ap()[:, k * QK:(k + 1) * QK])
            nc.sync.dma_start(xsb[k], xT.ap()[k * P:(k + 1) * P, :])
        for m in range(1, NMQ):
            nc.sync.dma_start(wqt[m], Wq.ap()[m])
            nc.sync.dma_start(wkt[m], Wk.ap()[m])
        for k in range(NMQ):
            nc.sync.dma_start(wot[k], Wo.ap()[k * P:(k + 1) * P, :])

        def evict_v(tb, ps):
            vdst = vsb[tb].rearrange("p (g w) -> p g w", w=65)
            vsrc = ps.rearrange("p (g w) -> p g w", w=64)
            nc.vector.tensor_copy(vdst[:, :, 0:64], vsrc[:, :, :])
            nc.vector.memset(vdst[:, :, 64:65], 1.0)

        def evict_qk(dst, ps, bias, m, ch):
            nc.vector.tensor_scalar_add(
                dst[m][:, ch * 512:(ch + 1) * 512], ps, bias[:, m:m + 1])

        # ================= prefix: v(t0..3) + q/k(m0, cols 0:512) ========
        # v first, k-outer, paced by the x/wv chunk DMAs; q/k m0 after
        # (x is resident by then), so the in-order tensor queue never
        # blocks on a weight DMA that arrives late.
        pv = [pp.tile([P, 512], FP, name=f"pv{tb}",
                      tag="po" if tb < 2 else "fps", bufs=2)
              for tb in range(4)]
        pq0 = pp.tile([P, 1024], FP, name="pq0", tag="wide", bufs=2)
        pk0 = pp.tile([P, 1024], FP, name="pk0", tag="wide", bufs=2)
        # PE warmup: ~5us of matmuls on a zeroed tile while the first DMAs
        # land, so HAM un-throttles (1.2 -> 2.4 GHz) before real work.
        zt = ep.tile([P, 512], BF, name="warm", tag="warm", bufs=1)
        nc.vector.memset(zt, 0.0)
        nc.vector.memset(ones64, 1.0)
        for _ in range(24):
            nc.tensor.matmul(pq0[:, 0:512], zt[:, 0:P], zt,
                             start=True, stop=True)
        # q/k m0 covers BOTH chunks 0 and 1 here (pq0/pk0 are 1024 wide):
        # the extra per-k matmuls absorb the DMA pacing gaps in the
        # prefix, where no other fill material exists yet.
        for k in range(NKT):
            st, sp = (k == 0), (k == NKT - 1)
            for tb in range(4):
                nc.tensor.matmul(
                    pv[tb], xsb[k][:, tb * P:(tb + 1) * P],
                    wvt[:, k * QK:(k + 1) * QK], start=st, stop=sp)
            nc.tensor.matmul(pq0[:, 0:512], wqt[0][:, k * P:(k + 1) * P],
                             xsb[k][:, 0:512], start=st, stop=sp)
            nc.tensor.matmul(pk0[:, 0:512], wkt[0][:, k * P:(k + 1) * P],
                             xsb[k][:, 0:512], start=st, stop=sp)
            nc.tensor.matmul(pq0[:, 512:1024], wqt[0][:, k * P:(k + 1) * P],
                             xsb[k][:, 512:1024], start=st, stop=sp)
            nc.tensor.matmul(pk0[:, 512:1024], wkt[0][:, k * P:(k + 1) * P],
                             xsb[k][:, 512:1024], start=st, stop=sp)
        for tb in range(4):
            evict_v(tb, pv[tb])
        evict_qk(qT, pq0[:, 0:512], bqt, 0, 0)
        evict_qk(kT, pk0[:, 0:512], bkt, 0, 0)
        evict_qk(qT, pq0[:, 512:1024], bqt, 0, 1)
        evict_qk(kT, pk0[:, 512:1024], bkt, 0, 1)

        # ================= fill-item machinery ===========================
        # Two pools of fill work. DL holds deadline items (the v/q/k
        # projections, consumed by a later q-chunk's attention) behind a
        # cursor, with named marks for drain points. FF holds pure filler
        # (out-projection tiles with no deadline). pump() prefers deadline
        # work and falls back to filler, so filler survives for the late
        # q-chunks and the final-pair denominator chain, where an empty
        # queue would leave the PE micro-idling on the ACT WAR wait and
        # set off HAM re-throttle oscillation.
        DL = []
        DLMARK = {}
        dl_pos = [0]
        FF = deque()

        def qk_chunk(which, m, ch):
            wt = wqt[m] if which == "q" else wkt[m]
            dst = qT if which == "q" else kT
            bias = bqt if which == "q" else bkt
            holder = []
            for k in range(NKT):
                def mm(k=k, holder=holder, wt=wt, m=m, ch=ch, which=which):
                    if k == 0:
                        holder.append(pp.tile(
                            [P, 512], FP, name=f"f{which}{m}{ch}",
                            tag="fps", bufs=2))
                    nc.tensor.matmul(
                        holder[0], wt[:, k * P:(k + 1) * P],
                        xsb[k][:, ch * 512:(ch + 1) * 512],
                        start=(k == 0), stop=(k == NKT - 1))
                DL.append(mm)

            def ev(holder=holder, dst=dst, bias=bias, m=m, ch=ch):
                evict_qk(dst, holder[0], bias, m, ch)
            DL.append(ev)

        def v_chunk(tb):
            holder = []
            for k in range(NKT):
                def mm(k=k, holder=holder, tb=tb):
                    if k == 0:
                        holder.append(pp.tile(
                            [P, 512], FP, name=f"fv{tb}", tag="fps", bufs=2))
                    nc.tensor.matmul(
                        holder[0], xsb[k][:, tb * P:(tb + 1) * P],
                        wvt[:, k * QK:(k + 1) * QK],
                        start=(k == 0), stop=(k == NKT - 1))
                DL.append(mm)

            def ev(holder=holder, tb=tb):
                evict_v(tb, holder[0])
            DL.append(ev)

        def op_half(t, n, korder=(0, 1, 2, 3), kmax=NMQ, tag="fps", bufs=2):
            # out-projection tile half: a partial that accumulates the
            # first kmax m-tiles of korder, and a finisher that runs the
            # remainder plus the eviction (idempotent on the partial).
            state = {"ps": None, "i": 0}

            def mm_to(ilim):
                if state["ps"] is None:
                    state["ps"] = pp.tile(
                        [P, 512], FP, name=f"fy{t}{n}", tag=tag, bufs=bufs)
                while state["i"] < ilim:
                    i = state["i"]
                    k = korder[i]
                    nc.tensor.matmul(
                        state["ps"], aT[k][:, t * P:(t + 1) * P],
                        wot[k][:, n * 512:(n + 1) * 512],
                        start=(i == 0), stop=(i == NMQ - 1))
                    state["i"] += 1

            def partial():
                mm_to(kmax)

            def finish():
                mm_to(NMQ)
                ye = ep.tile([P, 512], BF, name=f"ye{t}{n}", tag="ye",
                             bufs=3)
                nc.vector.tensor_copy(ye, state["ps"])
                nc.sync.dma_start(
                    y.ap()[t * P:(t + 1) * P, n * 512:(n + 1) * 512], ye)
            return partial, finish

        def outproj_tile(t):
            for n in range(2):
                holder = []
                for k in range(NMQ):
                    def mm(k=k, holder=holder, t=t, n=n):
                        if k == 0:
                            holder.append(pp.tile(
                                [P, 512], FP, name=f"fy{t}{n}",
                                tag="fps", bufs=2))
                        nc.tensor.matmul(
                            holder[0], aT[k][:, t * P:(t + 1) * P],
                            wot[k][:, n * 512:(n + 1) * 512],
                            start=(k == 0), stop=(k == NMQ - 1))
                    FF.append(mm)

                def ev(holder=holder, t=t, n=n):
                    ye = ep.tile([P, 512], BF, name=f"ye{t}{n}", tag="ye",
                                 bufs=3)
                    nc.vector.tensor_copy(ye, holder[0])
                    nc.sync.dma_start(
                        y.ap()[t * P:(t + 1) * P, n * 512:(n + 1) * 512], ye)
                FF.append(ev)

        def pump(n):
            while n > 0:
                if dl_pos[0] < len(DL):
                    DL[dl_pos[0]]()
                    dl_pos[0] += 1
                elif FF:
                    FF.popleft()()
                else:
                    return
                n -= 1

        def mark(label):
            DLMARK[label] = len(DL)

        def drain_to(label):
            end = DLMARK[label]
            while dl_pos[0] < end:
                DL[dl_pos[0]]()
                dl_pos[0] += 1

        def drain_all():
            while dl_pos[0] < len(DL):
                DL[dl_pos[0]]()
                dl_pos[0] += 1
            while FF:
                FF.popleft()()

        def queue_vqk(qc, skip_m0=False):
            for tb in range(4 * qc, 4 * qc + 4):
                v_chunk(tb)
            for m in range(NMQ):
                if skip_m0 and m == 0:
                    continue  # prefix already produced q/k m0 chunk 1
                qk_chunk("q", m, qc)
                qk_chunk("k", m, qc)
            mark(f"qc{qc}")

        for m in range(1, NMQ):
            qk_chunk("q", m, 0)
            qk_chunk("k", m, 0)
            mark(f"m{m}c0")
        queue_vqk(1, skip_m0=True)

        # ================= attention =====================================
        def attention_pair(qc, hp, fpump=2, tail_fills=None):
            c0 = qc * 512
            vle, vlo = 65 * (2 * hp), 65 * (2 * hp + 1)
            po_e = pp.tile([P, 512], FP, name=f"poe{qc}{hp}", tag="po",
                           bufs=2)
            po_o = pp.tile([P, 512], FP, name=f"poo{qc}{hp}", tag="po",
                           bufs=2)
            jmax = 4 * qc + 3

            def flush_av(pend):
                spans, es_e, es_o = pend
                for j, lo, d, w in spans:
                    nc.tensor.matmul(
                        po_e[0:65, lo:lo + w], vsb[j][:, vle:vle + 65],
                        es_e[:, d:d + w],
                        start=(j == 0), stop=(j == jmax))
                for j, lo, d, w in spans:
                    nc.tensor.matmul(
                        po_o[0:65, lo:lo + w], vsb[j][:, vlo:vlo + 65],
                        es_o[:, d:d + w],
                        start=(j == 0), stop=(j == jmax))

            # two-deep software pipeline: AV consumes the exp from two
            # steps back, so the in-order tensor queue never parks on ACT.
            pends = deque()
            for pr in range(2 * qc + 2):
                spans = []
                dst = 0
                for i in range(2):
                    j = 2 * pr + i
                    r = j - 4 * qc
                    lo = 0 if r < 0 else 128 * r
                    w = 512 - lo
                    dst = max(dst, i * 512 if r < 1 else 0)
                    spans.append((j, lo, dst, w))
                    dst += w
                pa, pb, pc = PUMP[qc]
                pump(pa)
                if len(pends) >= 2:
                    flush_av(pends.popleft())
                pump(pb)
                ps_e = pp.tile([P, 1024], FP, name=f"pse{qc}{hp}{pr}",
                               tag="wide", bufs=2)
                ps_o = pp.tile([P, 1024], FP, name=f"pso{qc}{hp}{pr}",
                               tag="wide", bufs=2)
                es_e = ep.tile([P, 1024], BF, name=f"ese{qc}{hp}{pr}",
                               tag="es", bufs=8)
                es_o = ep.tile([P, 1024], BF, name=f"eso{qc}{hp}{pr}",
                               tag="es", bufs=8)
                e0 = spans[0][2]
                e1 = spans[1][2] + spans[1][3]
                # row-tiled: even head on PE rows 0:64, odd on 64:128.
                # The e-group matmuls stall on the previous step's es_e
                # activation (wide-PSUM WAR) and the o-group on es_o,
                # which completes ~one activation later — so fills are
                # pumped between the groups to absorb that window, and
                # each activation is issued immediately after its group
                # so the ACT pipeline starts as early as possible.
                for j, lo, d, w in spans:
                    nc.tensor.matmul(
                        ps_e[:, d:d + w], kT[hp][0:64, j * P:(j + 1) * P],
                        qT[hp][0:64, c0 + lo:c0 + 512],
                        start=True, stop=True)
                nc.scalar.activation(es_e[:, e0:e1], ps_e[:, e0:e1], EXP,
                                     scale=SCALE)
                pump(pc)
                for j, lo, d, w in spans:
                    nc.tensor.matmul(
                        ps_o[:, d:d + w], kT[hp][64:128, j * P:(j + 1) * P],
                        qT[hp][64:128, c0 + lo:c0 + 512],
                        start=True, stop=True)
                nc.scalar.activation(es_o[:, e0:e1], ps_o[:, e0:e1], EXP,
                                     scale=SCALE)
                for j, lo, d, w in spans:
                    if j - 4 * qc >= 0:
                        nc.vector.tensor_mul(
                            es_e[:, d:d + 128], es_e[:, d:d + 128], trit)
                        nc.vector.tensor_mul(
                            es_o[:, d:d + 128], es_o[:, d:d + 128], trit)
                pends.append((spans, es_e, es_o))
            while pends:
                pump(fpump)
                flush_av(pends.popleft())

            # denominator row 64 -> 1/den -> broadcast -> normalize the
            # numerator straight out of PSUM into aT. Fill matmuls are
            # pumped between the steps so the PE never idles under this
            # latency chain. On the final pair the fill queues are dry,
            # so the broadcast runs as a K=1 matmul on the (idle) PE
            # into a free wide PSUM slot, and a held-back out-projection
            # partial is injected into the chain.
            den_e = ep.tile([1, 512], FP, name=f"dne{qc}{hp}", tag="dn",
                            bufs=4)
            den_o = ep.tile([1, 512], FP, name=f"dno{qc}{hp}", tag="dn",
                            bufs=4)
            nc.vector.tensor_copy(den_e, po_e[64:65, :])
            nc.vector.tensor_copy(den_o, po_o[64:65, :])
            if tail_fills:
                tail_fills.pop(0)()
            pump(2)
            rs = ep.tile([1, 1024], FP, name=f"rs{qc}{hp}", tag="rs",
                         bufs=4)
            nc.vector.reciprocal_approx_fast(out=rs[:, 0:512], in_=den_e)
            nc.vector.reciprocal_approx_fast(out=rs[:, 512:1024], in_=den_o)
            pump(2)
            bcs = ep.tile([64, 1024], FP, name=f"bc{qc}{hp}", tag="bcs",
                          bufs=4)
            nc.gpsimd.partition_broadcast(bcs, rs)
            if tail_fills:
                tail_fills.pop(0)()
            pump(2)
            pump(2)
            nc.vector.tensor_mul(aT[hp][0:64, c0:c0 + 512],
                                 po_e[0:64, :], bcs[:, 0:512])
            nc.vector.tensor_mul(aT[hp][64:128, c0:c0 + 512],
                                 po_o[0:64, :], bcs[:, 512:1024])

        # qc=3 runs its pairs in order [1,2,3,0]: after the first three,
        # the out-projection of t12..15 can accumulate m-tiles 1..3, so
        # fresh fill material becomes available exactly when the regular
        # fill queues run dry — during the (long) final pair and its
        # denominator chain. The m0 matmul + eviction follow the final
        # pair's normalize.
        KORD3 = (1, 2, 3, 0)
        for qc in range(NQC):
            if qc > 0:
                drain_to(f"qc{qc}")
            hps = (1, 2, 3, 0) if qc == NQC - 1 else range(NHP)
            finals = []
            for pi, hp in enumerate(hps):
                if qc == 0 and hp < NHP - 1:
                    # prefetch-drain the NEXT pair's q/k chunks so their
                    # DVE evictions complete during this pair's attention
                    drain_to(f"m{hp + 1}c0")
                tf = None
                if qc == NQC - 1 and pi == NHP - 1:
                    # t12 partials ride the fill queue tail (fps slots
                    # stay held until their finals, so only this one
                    # tile's halves may be outstanding). One more
                    # partial goes into the denominator chain via the
                    # second wide PSUM slot.
                    for n in range(2):
                        partial, finish = op_half(12, n, korder=KORD3,
                                                  kmax=NMQ - 1)
                        FF.append(partial)
                        finals.append(finish)
                    for n in range(2):
                        partial, finish = op_half(13, n, korder=KORD3,
                                                  kmax=NMQ - 1, tag="wide")
                        (tf := tf if tf is not None else []).append(partial)
                        finals.append(finish)
                attention_pair(qc, hp, fpump=2, tail_fills=tf)
            for fin in finals:
                fin()
            if qc < NQC - 1:
                for t in range(4 * qc, 4 * qc + 4):
                    outproj_tile(t)
            else:
                for t, n in ((14, 0), (14, 1), (15, 0), (15, 1)):
                    partial, finish = op_half(t, n, korder=KORD3)
                    FF.append(partial)
                    FF.append(finish)
            if qc + 2 <= NQC - 1:
                queue_vqk(qc + 2)

        drain_all()

    nc.compile()
    _NC_CACHE["nc"] = nc
    return nc


def kernel(x, W_qkv, b_qkv, W_out, b_out):
    global _LAST_IN_MAPS
    x = np.asarray(x, dtype=np.float32)
    W_qkv = np.asarray(W_qkv, dtype=np.float32)
    b_qkv = np.asarray(b_qkv, dtype=np.float32)
    W_out = np.asarray(W_out, dtype=np.float32)
    b_out = np.asarray(b_out, dtype=np.float32)
    import ml_dtypes

    bf16 = ml_dtypes.bfloat16
    tri = np.triu(np.ones((P, P), dtype=np.float32)).astype(bf16)
    in_maps = []
    for c in range(NCORES):
        b, hg = c // 2, c % 2
        cols = slice(hg * QK, (hg + 1) * QK)
        wq = W_qkv[:, 0 * C:1 * C][:, cols]
        wk = W_qkv[:, 1 * C:2 * C][:, cols]
        wv = W_qkv[:, 2 * C:3 * C][:, cols]
        in_maps.append({
            "xT": np.ascontiguousarray(x[b].T).astype(bf16),
            "Wq": np.ascontiguousarray(
                wq.reshape(NKT, P, NMQ, P).transpose(2, 1, 0, 3)
                .reshape(NMQ, P, NKT * P)).astype(bf16),
            "Wk": np.ascontiguousarray(
                wk.reshape(NKT, P, NMQ, P).transpose(2, 1, 0, 3)
                .reshape(NMQ, P, NKT * P)).astype(bf16),
            "Wv": np.ascontiguousarray(
                wv.reshape(NKT, P, QK).transpose(1, 0, 2)
                .reshape(P, NKT * QK)).astype(bf16),
            "bq": np.ascontiguousarray(b_qkv[0 * C:1 * C][cols, None]),
            "bk": np.ascontiguousarray(b_qkv[1 * C:2 * C][cols, None]),
            "Wo": np.ascontiguousarray(W_out[hg * QK:(hg + 1) * QK, :]).astype(bf16),
            "tri": tri,
        })
    _LAST_IN_MAPS = in_maps
    nc = build_nc()
    res = run_bass_kernel_spmd(nc, in_maps, core_ids=list(range(NCORES)))
    # v-bias and output bias are affine in the output: softmax rows sum to 1.
    extra = b_qkv[2 * C:3 * C] @ W_out + b_out
    out = np.empty((B, T, C), dtype=np.float32)
    for b in range(B):
        out[b] = (res.results[2 * b]["y"].astype(np.float32)
                  + res.results[2 * b + 1]["y"].astype(np.float32) + extra)
    return out



# revision 45
# speedup vs baseline: 1.0175x; 1.0175x over previous
"""Causal self-attention (B=4, T=2048, C=1024, H=16) on 8 TRN2 NeuronCores.

Sharding: core c handles batch b = c//2 and heads [8*(c%2), 8*(c%2)+8).
Each core computes the qkv projection for its 8 heads, flash-style causal
attention, and a partial output projection (its heads' slice of W_out rows).
Host sums the two partials per batch and adds the bias terms (v/out biases
are affine in the output because softmax rows sum to 1).

Schedule (single fused stream, tensor engine never drains):
  prefix   x arrives k-tile by k-tile; v(t0..3)/q(m0)/k(m0) projections run
           k-outer, pipelined against the DMA.
  steady   attention runs per (q-chunk, head-pair). The two heads of a pair
           occupy partitions 0:64 / 64:128, so their K=64 score matmuls
           land on disjoint PE row-groups and execute concurrently (row
           tiling). Remaining qkv projections and the output projection are
           chopped into single-matmul "fill" items, pumped between
           attention ops so the PE stays busy while ACT does the exps.
  denom    the ones-column in v makes the AV matmul emit the softmax
           denominator; 1/den via DVE reciprocal_approx_fast straight from
           PSUM, broadcast on gpsimd, one DVE multiply into aT.
All matmuls bf16 with fp32 PSUM accumulation; output y returned bf16
(partials are summed on host in fp32).
"""
from collections import deque

import numpy as np
import concourse.bass as bass  # noqa: F401  (registers engines)
import concourse.mybir as mybir
import concourse.tile as tile
from concourse import bacc
from concourse.bass_utils import run_bass_kernel_spmd

# problem constants (hardcoded per contract)
B, T, C, H, D = 4, 2048, 1024, 16, 64
NCORES = 8
NH = H // 2          # heads per core = 8
NHP = NH // 2        # head pairs per core = 4
QK = NH * D          # 512 qkv cols per core per q/k/v
SCALE = float(D) ** -0.5
P = 128
NKT = C // P         # 8 contraction tiles for the projections
NMQ = QK // P        # 4 row-tiles of qT/kT
NTT = T // P         # 16 t-blocks
NQC = T // 512       # 4 q-chunks
FP = mybir.dt.float32
BF = mybir.dt.bfloat16
EXP = mybir.ActivationFunctionType.Exp

# fill matmuls per attention pair-step: (before AV flush, after AV flush,
# between the even/odd score groups). Tuned so the fill supply lasts
# through qc=3 (over-pumping early starves the late pairs into HAM
# re-throttle oscillation).
PUMP = {0: (1, 0, 1), 1: (1, 0, 2), 2: (1, 0, 2), 3: (2, 1, 2)}

_NC_CACHE = {}
_LAST_IN_MAPS = None


def build_nc():
    if "nc" in _NC_CACHE:
        return _NC_CACHE["nc"]
    nc = bacc.Bacc(target_bir_lowering=False)
    xT = nc.declare_dram_parameter("xT", [C, T], BF, isOutput=False)
    Wq = nc.declare_dram_parameter("Wq", [NMQ, P, NKT * P], BF, isOutput=False)
    Wk = nc.declare_dram_parameter("Wk", [NMQ, P, NKT * P], BF, isOutput=False)
    Wv = nc.declare_dram_parameter("Wv", [P, NKT * QK], BF, isOutput=False)
    bq = nc.declare_dram_parameter("bq", [QK, 1], FP, isOutput=False)
    bk = nc.declare_dram_parameter("bk", [QK, 1], FP, isOutput=False)
    Wo = nc.declare_dram_parameter("Wo", [QK, C], BF, isOutput=False)
    tri = nc.declare_dram_parameter("tri", [P, P], BF, isOutput=False)
    y = nc.declare_dram_parameter("y", [T, C], BF, isOutput=True)

    with nc.allow_low_precision(reason="bf16 attention"), \
         tile.TileContext(nc) as tc, \
         tc.tile_pool(name="persist", bufs=1) as pers, \
         tc.tile_pool(name="psum", bufs=1, space="PSUM") as pp, \
         tc.tile_pool(name="evict", bufs=1) as ep:

        # ---- persistent tiles
        xsb = [pers.tile([P, T], BF, name=f"x{k}", tag=f"x{k}")
               for k in range(NKT)]
        qT = [pers.tile([P, T], BF, name=f"qT{m}", tag=f"qT{m}")
              for m in range(NMQ)]
        kT = [pers.tile([P, T], BF, name=f"kT{m}", tag=f"kT{m}")
              for m in range(NMQ)]
        aT = [pers.tile([P, T], BF, name=f"aT{m}", tag=f"aT{m}")
              for m in range(NMQ)]
        vsb = [pers.tile([P, NH * 65], BF, name=f"v{t}", tag=f"v{t}")
               for t in range(NTT)]
        wqt = [pers.tile([P, NKT * P], BF, name=f"wq{m}", tag=f"wq{m}")
               for m in range(NMQ)]
        wkt = [pers.tile([P, NKT * P], BF, name=f"wk{m}", tag=f"wk{m}")
               for m in range(NMQ)]
        wvt = pers.tile([P, NKT * QK // P * P], BF, name="wv", tag="wv")
        wot = [pers.tile([P, C], BF, name=f"wo{k}", tag=f"wo{k}")
               for k in range(NMQ)]
        trit = pers.tile([P, P], BF, name="trit", tag="trit")
        bqt = pers.tile([P, NMQ], FP, name="bqt", tag="bqt")
        bkt = pers.tile([P, NMQ], FP, name="bkt", tag="bkt")
        ones64 = pers.tile([1, 64], FP, name="ones64", tag="ones64")

        # ---- DMAs, in the order the prefix consumes them: wv/x chunk k
        # arrive together so the k-outer v matmuls start within ~2us.
        nc.sync.dma_start(trit, tri.ap())
        nc.sync.dma_start(bqt, bq.ap().rearrange("(m p) o -> p (m o)", p=P))
        nc.sync.dma_start(bkt, bk.ap().rearrange("(m p) o -> p (m o)", p=P))
        nc.sync.dma_start(wqt[0], Wq.ap()[0])
        nc.sync.dma_start(wkt[0], Wk.ap()[0])
        # x arrives in halves: the prefix (v t0..3, q/k m0 ch0/ch1) only
        # touches t-columns 0:1024, so the first halves ship first and
        # the prefix-critical DMA volume is halved; second halves land
        # during qc0/qc1 attention, well before the qc2/qc3 fills that
        # consume them.
        for k in range(NKT):
            nc.sync.dma_start(wvt[:, k * QK:(k + 1) * QK],
                              Wv.ap()[:, k * QK:(k + 1) * QK])
            nc.sync.dma_start(xsb[k][:, 0:1024],
                              xT.ap()[k * P:(k + 1) * P, 0:1024])
        for m in range(1, NMQ):
            nc.sync.dma_start(wqt[m], Wq.ap()[m])
            nc.sync.dma_start(wkt[m], Wk.ap()[m])
        for k in range(NKT):
            nc.sync.dma_start(xsb[k][:, 1024:2048],
                              xT.ap()[k * P:(k + 1) * P, 1024:2048])
        for k in range(NMQ):
            nc.sync.dma_start(wot[k], Wo.ap()[k * P:(k + 1) * P, :])

        def evict_v(tb, ps):
            vdst = vsb[tb].rearrange("p (g w) -> p g w", w=65)
            vsrc = ps.rearrange("p (g w) -> p g w", w=64)
            nc.vector.tensor_copy(vdst[:, :, 0:64], vsrc[:, :, :])
            nc.vector.memset(vdst[:, :, 64:65], 1.0)

        def evict_qk(dst, ps, bias, m, ch):
            nc.vector.tensor_scalar_add(
                dst[m][:, ch * 512:(ch + 1) * 512], ps, bias[:, m:m + 1])

        # ================= prefix: v(t0..3) + q/k(m0, cols 0:512) ========
        # v first, k-outer, paced by the x/wv chunk DMAs; q/k m0 after
        # (x is resident by then), so the in-order tensor queue never
        # blocks on a weight DMA that arrives late.
        pv = [pp.tile([P, 512], FP, name=f"pv{tb}",
                      tag="po" if tb < 2 else "fps", bufs=2)
              for tb in range(4)]
        pq0 = pp.tile([P, 1024], FP, name="pq0", tag="wide", bufs=2)
        pk0 = pp.tile([P, 1024], FP, name="pk0", tag="wide", bufs=2)
        # PE warmup: ~5us of matmuls on a zeroed tile while the first DMAs
        # land, so HAM un-throttles (1.2 -> 2.4 GHz) before real work.
        zt = ep.tile([P, 512], BF, name="warm", tag="warm", bufs=1)
        nc.vector.memset(zt, 0.0)
        nc.vector.memset(ones64, 1.0)
        for _ in range(24):
            nc.tensor.matmul(pq0[:, 0:512], zt[:, 0:P], zt,
                             start=True, stop=True)
        # q/k m0 covers BOTH chunks 0 and 1 here (pq0/pk0 are 1024 wide):
        # the extra per-k matmuls absorb the DMA pacing gaps in the
        # prefix, where no other fill material exists yet.
        for k in range(NKT):
            st, sp = (k == 0), (k == NKT - 1)
            for tb in range(4):
                nc.tensor.matmul(
                    pv[tb], xsb[k][:, tb * P:(tb + 1) * P],
                    wvt[:, k * QK:(k + 1) * QK], start=st, stop=sp)
            nc.tensor.matmul(pq0[:, 0:512], wqt[0][:, k * P:(k + 1) * P],
                             xsb[k][:, 0:512], start=st, stop=sp)
            nc.tensor.matmul(pk0[:, 0:512], wkt[0][:, k * P:(k + 1) * P],
                             xsb[k][:, 0:512], start=st, stop=sp)
            nc.tensor.matmul(pq0[:, 512:1024], wqt[0][:, k * P:(k + 1) * P],
                             xsb[k][:, 512:1024], start=st, stop=sp)
            nc.tensor.matmul(pk0[:, 512:1024], wkt[0][:, k * P:(k + 1) * P],
                             xsb[k][:, 512:1024], start=st, stop=sp)
        for tb in range(4):
            evict_v(tb, pv[tb])
        evict_qk(qT, pq0[:, 0:512], bqt, 0, 0)
        evict_qk(kT, pk0[:, 0:512], bkt, 0, 0)
        evict_qk(qT, pq0[:, 512:1024], bqt, 0, 1)
        evict_qk(kT, pk0[:, 512:1024], bkt, 0, 1)

        # ================= fill-item machinery ===========================
        # Two pools of fill work. DL holds deadline items (the v/q/k
        # projections, consumed by a later q-chunk's attention) behind a
        # cursor, with named marks for drain points. FF holds pure filler
        # (out-projection tiles with no deadline). pump() prefers deadline
        # work and falls back to filler, so filler survives for the late
        # q-chunks and the final-pair denominator chain, where an empty
        # queue would leave the PE micro-idling on the ACT WAR wait and
        # set off HAM re-throttle oscillation.
        DL = []
        DLMARK = {}
        dl_pos = [0]
        FF = deque()

        def qk_chunk(which, m, ch):
            wt = wqt[m] if which == "q" else wkt[m]
            dst = qT if which == "q" else kT
            bias = bqt if which == "q" else bkt
            holder = []
            for k in range(NKT):
                def mm(k=k, holder=holder, wt=wt, m=m, ch=ch, which=which):
                    if k == 0:
                        holder.append(pp.tile(
                            [P, 512], FP, name=f"f{which}{m}{ch}",
                            tag="fps", bufs=2))
                    nc.tensor.matmul(
                        holder[0], wt[:, k * P:(k + 1) * P],
                        xsb[k][:, ch * 512:(ch + 1) * 512],
                        start=(k == 0), stop=(k == NKT - 1))
                DL.append(mm)

            def ev(holder=holder, dst=dst, bias=bias, m=m, ch=ch):
                evict_qk(dst, holder[0], bias, m, ch)
            DL.append(ev)

        def v_chunk(tb):
            holder = []
            for k in range(NKT):
                def mm(k=k, holder=holder, tb=tb):
                    if k == 0:
                        holder.append(pp.tile(
                            [P, 512], FP, name=f"fv{tb}", tag="fps", bufs=2))
                    nc.tensor.matmul(
                        holder[0], xsb[k][:, tb * P:(tb + 1) * P],
                        wvt[:, k * QK:(k + 1) * QK],
                        start=(k == 0), stop=(k == NKT - 1))
                DL.append(mm)

            def ev(holder=holder, tb=tb):
                evict_v(tb, holder[0])
            DL.append(ev)

        def op_half(t, n, korder=(0, 1, 2, 3), kmax=NMQ, tag="fps", bufs=2):
            # out-projection tile half: a partial that accumulates the
            # first kmax m-tiles of korder, and a finisher that runs the
            # remainder plus the eviction (idempotent on the partial).
            state = {"ps": None, "i": 0}

            def mm_to(ilim):
                if state["ps"] is None:
                    state["ps"] = pp.tile(
                        [P, 512], FP, name=f"fy{t}{n}", tag=tag, bufs=bufs)
                while state["i"] < ilim:
                    i = state["i"]
                    k = korder[i]
                    nc.tensor.matmul(
                        state["ps"], aT[k][:, t * P:(t + 1) * P],
                        wot[k][:, n * 512:(n + 1) * 512],
                        start=(i == 0), stop=(i == NMQ - 1))
                    state["i"] += 1

            def partial():
                mm_to(kmax)

            def finish():
                mm_to(NMQ)
                ye = ep.tile([P, 512], BF, name=f"ye{t}{n}", tag="ye",
                             bufs=3)
                nc.vector.tensor_copy(ye, state["ps"])
                nc.sync.dma_start(
                    y.ap()[t * P:(t + 1) * P, n * 512:(n + 1) * 512], ye)
            return partial, finish

        def outproj_tile(t):
            for n in range(2):
                holder = []
                for k in range(NMQ):
                    def mm(k=k, holder=holder, t=t, n=n):
                        if k == 0:
                            holder.append(pp.tile(
                                [P, 512], FP, name=f"fy{t}{n}",
                                tag="fps", bufs=2))
                        nc.tensor.matmul(
                            holder[0], aT[k][:, t * P:(t + 1) * P],
                            wot[k][:, n * 512:(n + 1) * 512],
                            start=(k == 0), stop=(k == NMQ - 1))
                    FF.append(mm)

                def ev(holder=holder, t=t, n=n):
                    ye = ep.tile([P, 512], BF, name=f"ye{t}{n}", tag="ye",
                                 bufs=3)
                    nc.vector.tensor_copy(ye, holder[0])
                    nc.sync.dma_start(
                        y.ap()[t * P:(t + 1) * P, n * 512:(n + 1) * 512], ye)
                FF.append(ev)

        def pump(n):
            while n > 0:
                if dl_pos[0] < len(DL):
                    DL[dl_pos[0]]()
                    dl_pos[0] += 1
                elif FF:
                    FF.popleft()()
                else:
                    return
                n -= 1

        def mark(label):
            DLMARK[label] = len(DL)

        def drain_to(label):
            end = DLMARK[label]
            while dl_pos[0] < end:
                DL[dl_pos[0]]()
                dl_pos[0] += 1

        def drain_all():
            while dl_pos[0] < len(DL):
                DL[dl_pos[0]]()
                dl_pos[0] += 1
            while FF:
                FF.popleft()()

        def queue_vqk(qc, skip_m0=False):
            for tb in range(4 * qc, 4 * qc + 4):
                v_chunk(tb)
            for m in range(NMQ):
                if skip_m0 and m == 0:
                    continue  # prefix already produced q/k m0 chunk 1
                qk_chunk("q", m, qc)
                qk_chunk("k", m, qc)
            mark(f"qc{qc}")

        for m in range(1, NMQ):
            qk_chunk("q", m, 0)
            qk_chunk("k", m, 0)
            mark(f"m{m}c0")
        queue_vqk(1, skip_m0=True)

        # ================= attention =====================================
        def attention_pair(qc, hp, fpump=2, tail_fills=None):
            c0 = qc * 512
            vle, vlo = 65 * (2 * hp), 65 * (2 * hp + 1)
            po_e = pp.tile([P, 512], FP, name=f"poe{qc}{hp}", tag="po",
                           bufs=2)
            po_o = pp.tile([P, 512], FP, name=f"poo{qc}{hp}", tag="po",
                           bufs=2)
            jmax = 4 * qc + 3

            def flush_av(pend):
                spans, es_e, es_o = pend
                for j, lo, d, w in spans:
                    nc.tensor.matmul(
                        po_e[0:65, lo:lo + w], vsb[j][:, vle:vle + 65],
                        es_e[:, d:d + w],
                        start=(j == 0), stop=(j == jmax))
                for j, lo, d, w in spans:
                    nc.tensor.matmul(
                        po_o[0:65, lo:lo + w], vsb[j][:, vlo:vlo + 65],
                        es_o[:, d:d + w],
                        start=(j == 0), stop=(j == jmax))

            # two-deep software pipeline: AV consumes the exp from two
            # steps back, so the in-order tensor queue never parks on ACT.
            pends = deque()
            for pr in range(2 * qc + 2):
                spans = []
                dst = 0
                for i in range(2):
                    j = 2 * pr + i
                    r = j - 4 * qc
                    lo = 0 if r < 0 else 128 * r
                    w = 512 - lo
                    dst = max(dst, i * 512 if r < 1 else 0)
                    spans.append((j, lo, dst, w))
                    dst += w
                pa, pb, pc = PUMP[qc]
                pump(pa)
                if len(pends) >= 2:
                    flush_av(pends.popleft())
                pump(pb)
                ps_e = pp.tile([P, 1024], FP, name=f"pse{qc}{hp}{pr}",
                               tag="wide", bufs=2)
                ps_o = pp.tile([P, 1024], FP, name=f"pso{qc}{hp}{pr}",
                               tag="wide", bufs=2)
                es_e = ep.tile([P, 1024], BF, name=f"ese{qc}{hp}{pr}",
                               tag="es", bufs=8)
                es_o = ep.tile([P, 1024], BF, name=f"eso{qc}{hp}{pr}",
                               tag="es", bufs=8)
                e0 = spans[0][2]
                e1 = spans[1][2] + spans[1][3]
                # row-tiled: even head on PE rows 0:64, odd on 64:128.
                # The e-group matmuls stall on the previous step's es_e
                # activation (wide-PSUM WAR) and the o-group on es_o,
                # which completes ~one activation later — so fills are
                # pumped between the groups to absorb that window, and
                # each activation is issued immediately after its group
                # so the ACT pipeline starts as early as possible.
                for j, lo, d, w in spans:
                    nc.tensor.matmul(
                        ps_e[:, d:d + w], kT[hp][0:64, j * P:(j + 1) * P],
                        qT[hp][0:64, c0 + lo:c0 + 512],
                        start=True, stop=True)
                nc.scalar.activation(es_e[:, e0:e1], ps_e[:, e0:e1], EXP,
                                     scale=SCALE)
                pump(pc)
                for j, lo, d, w in spans:
                    nc.tensor.matmul(
                        ps_o[:, d:d + w], kT[hp][64:128, j * P:(j + 1) * P],
                        qT[hp][64:128, c0 + lo:c0 + 512],
                        start=True, stop=True)
                nc.scalar.activation(es_o[:, e0:e1], ps_o[:, e0:e1], EXP,
                                     scale=SCALE)
                for j, lo, d, w in spans:
                    if j - 4 * qc >= 0:
                        nc.vector.tensor_mul(
                            es_e[:, d:d + 128], es_e[:, d:d + 128], trit)
                        nc.vector.tensor_mul(
                            es_o[:, d:d + 128], es_o[:, d:d + 128], trit)
                pends.append((spans, es_e, es_o))
            while pends:
                pump(fpump)
                flush_av(pends.popleft())

            # denominator row 64 -> 1/den -> broadcast -> normalize the
            # numerator straight out of PSUM into aT. Fill matmuls are
            # pumped between the steps so the PE never idles under this
            # latency chain. On the final pair the fill queues are dry,
            # so the broadcast runs as a K=1 matmul on the (idle) PE
            # into a free wide PSUM slot, and a held-back out-projection
            # partial is injected into the chain.
            den_e = ep.tile([1, 512], FP, name=f"dne{qc}{hp}", tag="dn",
                            bufs=4)
            den_o = ep.tile([1, 512], FP, name=f"dno{qc}{hp}", tag="dn",
                            bufs=4)
            nc.vector.tensor_copy(den_e, po_e[64:65, :])
            nc.vector.tensor_copy(den_o, po_o[64:65, :])
            if tail_fills:
                tail_fills.pop(0)()
            pump(2)
            rs = ep.tile([1, 1024], FP, name=f"rs{qc}{hp}", tag="rs",
                         bufs=4)
            nc.vector.reciprocal_approx_fast(out=rs[:, 0:512], in_=den_e)
            nc.vector.reciprocal_approx_fast(out=rs[:, 512:1024], in_=den_o)
            pump(2)
            bcs = ep.tile([64, 1024], FP, name=f"bc{qc}{hp}", tag="bcs",
                          bufs=4)
            nc.gpsimd.partition_broadcast(bcs, rs)
            if tail_fills:
                tail_fills.pop(0)()
            pump(2)
            pump(2)
            if tail_fills is not None:
                # final pair: every fill source is exhausted, so keep
                # the PE streaming on dummy matmuls under the broadcast
                # latency — a HAM MID-window of idle here would halve
                # the rate for the whole out-projection tail.
                pdead = pp.tile([P, 512], FP, name="pdead", tag="wide",
                                bufs=2)
                for _ in range(8):
                    nc.tensor.matmul(pdead, zt[:, 0:P], zt,
                                     start=True, stop=True)
            nc.vector.tensor_mul(aT[hp][0:64, c0:c0 + 512],
                                 po_e[0:64, :], bcs[:, 0:512])
            nc.vector.tensor_mul(aT[hp][64:128, c0:c0 + 512],
                                 po_o[0:64, :], bcs[:, 512:1024])

        # qc=3 runs its pairs in order [1,2,3,0]: after the first three,
        # the out-projection of t12..15 can accumulate m-tiles 1..3, so
        # fresh fill material becomes available exactly when the regular
        # fill queues run dry — during the (long) final pair and its
        # denominator chain. The m0 matmul + eviction follow the final
        # pair's normalize.
        KORD3 = (1, 2, 3, 0)
        for qc in range(NQC):
            if qc > 0:
                drain_to(f"qc{qc}")
            hps = (1, 2, 3, 0) if qc == NQC - 1 else range(NHP)
            finals = []
            for pi, hp in enumerate(hps):
                if qc == 0 and hp < NHP - 1:
                    # prefetch-drain the NEXT pair's q/k chunks so their
                    # DVE evictions complete during this pair's attention
                    drain_to(f"m{hp + 1}c0")
                tf = None
                if qc == NQC - 1 and pi == NHP - 1:
                    # t12 partials ride the fill queue tail (fps slots
                    # stay held until their finals, so only this one
                    # tile's halves may be outstanding). One more
                    # partial goes into the denominator chain via the
                    # second wide PSUM slot.
                    for n in range(2):
                        partial, finish = op_half(12, n, korder=KORD3,
                                                  kmax=NMQ - 1)
                        FF.append(partial)
                        finals.append(finish)
                    for n in range(2):
                        partial, finish = op_half(13, n, korder=KORD3,
                                                  kmax=NMQ - 1, tag="wide")
                        (tf := tf if tf is not None else []).append(partial)
                        finals.append(finish)
                attention_pair(qc, hp, fpump=2, tail_fills=tf)
            for fin in finals:
                fin()
            if qc < NQC - 1:
                for t in range(4 * qc, 4 * qc + 4):
                    outproj_tile(t)
            else:
                for t, n in ((14, 0), (14, 1), (15, 0), (15, 1)):
                    partial, finish = op_half(t, n, korder=KORD3)
                    FF.append(partial)
                    FF.append(finish)
            if qc + 2 <= NQC - 1:
                queue_vqk(qc + 2)

        drain_all()

    nc.compile()
    _NC_CACHE["nc"] = nc
    return nc


def kernel(x, W_qkv, b_qkv, W_out, b_out):
    global _LAST_IN_MAPS
    x = np.asarray(x, dtype=np.float32)
    W_qkv = np.asarray(W_qkv, dtype=np.float32)
    b_qkv = np.asarray(b_qkv, dtype=np.float32)
    W_out = np.asarray(W_out, dtype=np.float32)
    b_out = np.asarray(b_out, dtype=np.float32)
    import ml_dtypes

    bf16 = ml_dtypes.bfloat16
    tri = np.triu(np.ones((P, P), dtype=np.float32)).astype(bf16)
    in_maps = []
    for c in range(NCORES):
        b, hg = c // 2, c % 2
        cols = slice(hg * QK, (hg + 1) * QK)
        wq = W_qkv[:, 0 * C:1 * C][:, cols]
        wk = W_qkv[:, 1 * C:2 * C][:, cols]
        wv = W_qkv[:, 2 * C:3 * C][:, cols]
        in_maps.append({
            "xT": np.ascontiguousarray(x[b].T).astype(bf16),
            "Wq": np.ascontiguousarray(
                wq.reshape(NKT, P, NMQ, P).transpose(2, 1, 0, 3)
                .reshape(NMQ, P, NKT * P)).astype(bf16),
            "Wk": np.ascontiguousarray(
                wk.reshape(NKT, P, NMQ, P).transpose(2, 1, 0, 3)
                .reshape(NMQ, P, NKT * P)).astype(bf16),
            "Wv": np.ascontiguousarray(
                wv.reshape(NKT, P, QK).transpose(1, 0, 2)
                .reshape(P, NKT * QK)).astype(bf16),
            "bq": np.ascontiguousarray(b_qkv[0 * C:1 * C][cols, None]),
            "bk": np.ascontiguousarray(b_qkv[1 * C:2 * C][cols, None]),
            "Wo": np.ascontiguousarray(W_out[hg * QK:(hg + 1) * QK, :]).astype(bf16),
            "tri": tri,
        })
    _LAST_IN_MAPS = in_maps
    nc = build_nc()
    res = run_bass_kernel_spmd(nc, in_maps, core_ids=list(range(NCORES)))
    # v-bias and output bias are affine in the output: softmax rows sum to 1.
    extra = b_qkv[2 * C:3 * C] @ W_out + b_out
    out = np.empty((B, T, C), dtype=np.float32)
    for b in range(B):
        out[b] = (res.results[2 * b]["y"].astype(np.float32)
                  + res.results[2 * b + 1]["y"].astype(np.float32) + extra)
    return out

